# revision 3
# baseline (speedup 1.0000x reference)
"""Trainium2 Bass kernel for sliding-window unfold (im2col).

reference:  out = x[:, idx, :]  with idx[w, f] = w + f
  x:   [128, 4096, 4]  f32
  out: [128, 4065, 32, 4]  f32

Key structural fact: out[b, w] (= 32*4 = 128 floats = 512 B) is the
contiguous slice x[b].flat[4w : 4w + 128].  The whole problem is a
sliding-window byte replication; HBM write bandwidth is the roofline.

Measured on TRN2 (trace analysis):
  - a dma_start whose SBUF side spans EXACTLY 128 partitions is sprayed
    across all 16 SDMA engines (~26.6 GB/s each); other SBUF partition
    counts land on a single engine (~20 GB/s).
  - every DMA queue (2x HWDGE + SWDGE) dispatches descriptors at
    ~16 ns/descriptor, so a 128-row DMA costs ~2.05 us of queue time
    regardless of row size.  Small-row loads are dispatch-limited.
  - a DVE expand copy of [128, 3968] f32 takes ~5.1 us (ACT ~3.6 us);
    SDMA engines drain one batch's bulk store in ~4.7 us/engine.

Per batch b on each core (pure data parallel, 16 batches/core):
  1. one DMA loads a replicated tile X[128, 248]: partition p holds
     x[b].flat[124p : 124p+248] - everything windows 31p..31p+30 touch.
  2. the expand X -> Y[128, 3968] (Y[p, 128j+i] = X[p, 4j+i]) is split
     by windows across DVE (16) and ACT (15) running in parallel
     (~2.6 us) so the first bulk store starts as early as possible.
  3. one 128-partition SWDGE DMA stores Y to out[b] windows 0..3967
     (contiguous 15.5 KB runs per partition - full-rate descriptors).
  4. the 97 ragged tail windows (3968..4064) are written by one
     HBM->HBM DMA on a HWDGE queue: 97 descriptors reading 512 B
     overlapping runs straight from x, writing the contiguous 49664 B
     tail of out[b].  No tail loads, no SBUF tile, no duplicate bytes,
     and no overlap with the bulk store's output region.

Schedule: the SWDGE bulk-store stream is the makespan; every cycle an
SDMA engine idles waiting for store descriptors is lost.  X loads gate
expands gate bulk stores, so they dispatch FIRST on both HWDGE queues
(alternating); the dependency-free tail DMAs queue up behind them and
drain mid-stream.  gpsimd generates only the 16 bulk stores (2.05 us
gen per 4.7 us of engine work - always ahead once the first expand
lands).
"""

import numpy as np

from concourse import bacc, mybir, tile
from concourse.bass_utils import run_bass_kernel_spmd

N_CORES = 8
B_FULL = 128
B = B_FULL // N_CORES  # 16 batches per core
S = 4096
C = 4
F = 32
W = S - F + 1    # 4065
FL = F * C       # 128 floats per window
XB = S * C       # 16384 floats per batch of x
OB = W * FL      # 520320 floats per batch of out
WPP = 31         # windows per partition in the bulk store
NBULK = 128 * WPP          # 3968 bulk windows per batch
NTAIL = W - NBULK          # 97 tail windows
YROW = WPP * FL            # 3968 floats per partition row
XROW = (WPP - 1) * C + FL  # 248 floats of x per partition
WSPLIT = 16                # windows expanded on DVE (rest on ACT)

_cache = {}


def build_nc():
    nc = bacc.Bacc("TRN2", target_bir_lowering=False)
    x = nc.dram_tensor("x", [B, S, C], mybir.dt.float32, kind="ExternalInput")
    out = nc.dram_tensor("out", [B, W, F, C], mybir.dt.float32, kind="ExternalOutput")

    with tile.TileContext(nc) as tc:
        with (
            tc.tile_pool(name="xp", bufs=12) as xp,
            tc.tile_pool(name="yp", bufs=10) as yp,
        ):
            # -- X loads first on both HWDGE queues (they gate the bulk
            #    store stream); alternate queues so X_b lands ~2x sooner.
            Xs = []
            for b in range(B):
                X = xp.tile([128, XROW], mybir.dt.float32)
                src = x[:].copy()
                src.ap = mybir.VecI64Pair([[WPP * C, 128], [1, XROW]])
                src.offset = b * XB
                (nc.sync if b % 2 == 0 else nc.scalar).dma_start(out=X[:, :], in_=src)
                Xs.append(X)

            # -- tail windows: dependency-free HBM->HBM DMAs queued
            #    behind the X loads; they drain mid-stream.
            for b in range(B):
                srcT = x[:].copy()
                srcT.ap = mybir.VecI64Pair([[C, NTAIL], [1, FL]])
                srcT.offset = b * XB + NBULK * C
                dstT = out[:].copy()
                dstT.ap = mybir.VecI64Pair([[FL, NTAIL], [1, FL]])
                dstT.offset = b * OB + NBULK * FL
                (nc.scalar if b % 2 == 0 else nc.sync).dma_start(out=dstT, in_=srcT)

            # -- expand (split DVE || ACT) + bulk store pipeline.
            for b in range(B):
                X = Xs[b]
                Y = yp.tile([128, YROW], mybir.dt.float32)

                srcA = X[:].copy()
                srcA.ap = mybir.VecI64Pair([[XROW, 128], [C, WSPLIT], [1, FL]])
                srcA.offset = 0
                dstA = Y[:].copy()
                dstA.ap = mybir.VecI64Pair([[YROW, 128], [FL, WSPLIT], [1, FL]])
                dstA.offset = 0
                nc.vector.tensor_copy(out=dstA, in_=srcA)

                srcB = X[:].copy()
                srcB.ap = mybir.VecI64Pair([[XROW, 128], [C, WPP - WSPLIT], [1, FL]])
                srcB.offset = WSPLIT * C
                dstB = Y[:].copy()
                dstB.ap = mybir.VecI64Pair([[YROW, 128], [FL, WPP - WSPLIT], [1, FL]])
                dstB.offset = WSPLIT * FL
                nc.scalar.copy(out=dstB, in_=srcB)

                dst3 = out[:].copy()
                dst3.ap = mybir.VecI64Pair([[YROW, 128], [1, YROW]])
                dst3.offset = b * OB
                nc.gpsimd.dma_start(out=dst3, in_=Y[:, :])

    nc.finalize()
    return nc


def run_sharded(x: np.ndarray, trace: bool = False):
    """Shard batch across 8 cores, run, gather. Returns (out, raw results)."""
    if "nc" not in _cache:
        _cache["nc"] = build_nc()
    nc = _cache["nc"]

    x = np.ascontiguousarray(x, dtype=np.float32)
    in_maps = [{"x": x[i * B : (i + 1) * B]} for i in range(N_CORES)]
    res = run_bass_kernel_spmd(nc, in_maps, list(range(N_CORES)), trace=trace)
    out = np.concatenate([res.results[i]["out"] for i in range(N_CORES)], axis=0)
    return out, res


def kernel(x: np.ndarray) -> np.ndarray:
    out, _ = run_sharded(x, trace=False)
    return out


# revision 4
# speedup vs baseline: 1.1719x; 1.1719x over previous
"""Trainium2 Bass kernel for sliding-window unfold (im2col).

reference:  out = x[:, idx, :]  with idx[w, f] = w + f
  x:   [128, 4096, 4]  f32
  out: [128, 4065, 32, 4]  f32

out[b, w] (= 128 floats = 512 B) is the contiguous slice
x[b].flat[4w : 4w + 128]; HBM write bandwidth is the roofline.

Measured on TRN2 (trace analysis across runs):
  - a dma_start whose SBUF side spans EXACTLY 128 partitions is sprayed
    across all 16 SDMA engines (~26.6 GB/s each); DRAM->DRAM DMAs and
    other partition counts land entirely on ONE engine (E64).
  - every DMA queue (sync/scalar HWDGE + gpsimd SWDGE) dispatches
    descriptors at ~16 ns/descriptor; a 128-row DMA costs ~2.05 us of
    queue time regardless of row size.
  - expands are cheaper split: DVE 16-window half ~1.3 us + ACT
    15-window half ~1.9 us in parallel, vs 5.1/3.6 us monolithic.

Per batch b on each core (pure data parallel, 16 batches/core):
  1. load X[128, 248]: partition p holds x[b].flat[124p : 124p+248].
  2. expand X -> Y[128, 3968] (Y[p, 128j+i] = X[p, 4j+i]), split
     across DVE (16 windows) and ACT (15 windows) in parallel.
  3. store Y -> out[b] windows 0..3967 (15.5 KB runs per partition).
  4. tail windows 3937..4064 ride a 128-partition load+store pair
     (31 rows rewrite bulk output with identical bytes - fast path).

This revision tests two-stream stores: even-b bulk stores go through
gpsimd (SWDGE), odd-b bulk stores through the scalar HWDGE queue as
direct SBUF->DRAM DMAs, so store descriptors dispatch from two queues
in parallel.  All loads ride the sync queue (X first - they gate the
stores - then tails).  Tail stores ride gpsimd after the bulk gens.
"""

import numpy as np

from concourse import bacc, mybir, tile
from concourse.bass_utils import run_bass_kernel_spmd

N_CORES = 8
B_FULL = 128
B = B_FULL // N_CORES  # 16 batches per core
S = 4096
C = 4
F = 32
W = S - F + 1    # 4065
FL = F * C       # 128 floats per window
XB = S * C       # 16384 floats per batch of x
OB = W * FL      # 520320 floats per batch of out
WPP = 31         # windows per partition in the bulk store
NBULK = 128 * WPP          # 3968 bulk windows per batch
NTAIL = W - NBULK          # 97 tail windows
YROW = WPP * FL            # 3968 floats per partition row
XROW = (WPP - 1) * C + FL  # 248 floats of x per partition
WSPLIT = 16                # windows expanded on DVE (rest on ACT)

_cache = {}


def build_nc():
    nc = bacc.Bacc("TRN2", target_bir_lowering=False)
    x = nc.dram_tensor("x", [B, S, C], mybir.dt.float32, kind="ExternalInput")
    out = nc.dram_tensor("out", [B, W, F, C], mybir.dt.float32, kind="ExternalOutput")

    with tile.TileContext(nc) as tc:
        with (
            tc.tile_pool(name="xp", bufs=12) as xp,
            tc.tile_pool(name="yp", bufs=10) as yp,
            tc.tile_pool(name="tp", bufs=16) as tp,
        ):
            # -- all loads on the sync queue: X loads first (they gate
            #    the store streams), tail loads behind them.
            Xs = []
            for b in range(B):
                X = xp.tile([128, XROW], mybir.dt.float32)
                src = x[:].copy()
                src.ap = mybir.VecI64Pair([[WPP * C, 128], [1, XROW]])
                src.offset = b * XB
                nc.sync.dma_start(out=X[:, :], in_=src)
                Xs.append(X)

            TBs = []
            for b in range(B):
                TB = tp.tile([128, FL], mybir.dt.float32)
                srcT = x[:].copy()
                srcT.ap = mybir.VecI64Pair([[C, 128], [1, FL]])
                srcT.offset = b * XB + (NBULK - 31) * C
                nc.sync.dma_start(out=TB[:, :], in_=srcT)
                TBs.append(TB)

            # -- expand (split DVE || ACT) + two-stream bulk stores.
            for b in range(B):
                X = Xs[b]
                Y = yp.tile([128, YROW], mybir.dt.float32)

                srcA = X[:].copy()
                srcA.ap = mybir.VecI64Pair([[XROW, 128], [C, WSPLIT], [1, FL]])
                srcA.offset = 0
                dstA = Y[:].copy()
                dstA.ap = mybir.VecI64Pair([[YROW, 128], [FL, WSPLIT], [1, FL]])
                dstA.offset = 0
                nc.vector.tensor_copy(out=dstA, in_=srcA)

                srcB = X[:].copy()
                srcB.ap = mybir.VecI64Pair([[XROW, 128], [C, WPP - WSPLIT], [1, FL]])
                srcB.offset = WSPLIT * C
                dstB = Y[:].copy()
                dstB.ap = mybir.VecI64Pair([[YROW, 128], [FL, WPP - WSPLIT], [1, FL]])
                dstB.offset = WSPLIT * FL
                nc.scalar.copy(out=dstB, in_=srcB)

                dst3 = out[:].copy()
                dst3.ap = mybir.VecI64Pair([[YROW, 128], [1, YROW]])
                dst3.offset = b * OB
                (nc.gpsimd if b % 2 == 0 else nc.scalar).dma_start(out=dst3, in_=Y[:, :])

            # -- tail stores on gpsimd after the bulk gens; their loads
            #    landed long ago.
            for b in range(B):
                dstT = out[:].copy()
                dstT.ap = mybir.VecI64Pair([[FL, 128], [1, FL]])
                dstT.offset = b * OB + (NBULK - 31) * FL
                nc.gpsimd.dma_start(out=dstT, in_=TBs[b][:, :])

    nc.finalize()
    return nc


def run_sharded(x: np.ndarray, trace: bool = False):
    """Shard batch across 8 cores, run, gather. Returns (out, raw results)."""
    if "nc" not in _cache:
        _cache["nc"] = build_nc()
    nc = _cache["nc"]

    x = np.ascontiguousarray(x, dtype=np.float32)
    in_maps = [{"x": x[i * B : (i + 1) * B]} for i in range(N_CORES)]
    res = run_bass_kernel_spmd(nc, in_maps, list(range(N_CORES)), trace=trace)
    out = np.concatenate([res.results[i]["out"] for i in range(N_CORES)], axis=0)
    return out, res


def kernel(x: np.ndarray) -> np.ndarray:
    out, _ = run_sharded(x, trace=False)
    return out
